# revision 35
# baseline (speedup 1.0000x reference)
"""MobileMQA1D attention block on 8 Trainium2 NeuronCores.

Reference computation (B=4, C=512, L=2048, H=8, D=64):
    xp = x.T                     # (L, C) per batch
    q/k/v = xp @ W.T + b         # heads (H, L, D)
    attn  = softmax(q k^T / sqrt(D))
    out   = (attn @ v) reassembled -> @ Wo.T + bo
    y     = x + out.T            # (C, L) per batch

Sharding: 8 cores = 4 batches x 2 query-halves. Each core computes K/V
for its whole batch and Q/attention/out-proj for its 1024-query half.
No cross-core communication.

Channel-first layout ("transposed scores") so the softmax sum rides the
matmul contraction axis (augmented V row 64 = ones -> denominator).

Engine split per head pair (keeps all three engines off the serial
path): head a's exp is a Schraudolph bf16-bitcast approximation on
VectorE (one tensor_scalar: i16 = s*scale*2^7/ln2 + 127*2^7 - C);
head b's exp is exact on ScalarE. sc_a frees LATER than sc_b (DVE 1454
vs ACT 1114 ns), so emitting sc_a's matmuls first makes the scheduler
see both score tiles co-ready and compile the h0/h64 matmuls adjacent,
where the PE runs them concurrently in distinct row groups.

PSUM (8 banks): A: kq1 4+2 | B: vp 2 + sc 4 | C: kq2 2 + sc 4
              | D: sc 4 + ut 4 | tail: op 8
"""

import math
import sys

sys.path.insert(0, "/opt/trn_rl_repo")


import numpy as np

import concourse.bass as bass
import concourse.mybir as mybir
import concourse.tile as tile
from concourse import bacc
from concourse.bass import ds, ts
from concourse.bass_utils import run_bass_kernel_spmd

F32 = mybir.dt.float32
BF16 = mybir.dt.bfloat16
I16 = mybir.dt.int16
EXP = mybir.ActivationFunctionType.Exp
MULT = mybir.AluOpType.mult
ADD = mybir.AluOpType.add

B, C, L, H = 4, 512, 2048, 8
D = C // H
LQ = L // 2
SCALE = float(D) ** -0.5
NCORES = 8
NL = L // 128  # 16 key chunks
NCH = C // 128  # 4 channel chunks

ESB_BUFS = 38  # j0's 32 exp tiles buffered through phase B + j1 hoist + slack

# Schraudolph exp in bf16 bit patterns (HW rounds to nearest)
SCHRAU_A = float(SCALE * (2.0**7) / math.log(2.0))
SCHRAU_B = float(127.0 * 2.0**7 - 0.0438 * 2.0**7)


def build_nc():
    nc = bacc.Bacc("TRN2", target_bir_lowering=False, debug=False)

    xb_d = nc.dram_tensor("xb", [C, L], BF16, kind="ExternalInput")
    wqT_d = nc.dram_tensor("wqT", [128, NCH, C], BF16, kind="ExternalInput")
    wkT_d = nc.dram_tensor("wkT", [128, NCH, C], BF16, kind="ExternalInput")
    wvT_d = nc.dram_tensor("wvT", [128, NCH, C], BF16, kind="ExternalInput")
    woT_d = nc.dram_tensor("woT", [128, NCH, C], BF16, kind="ExternalInput")
    xqr_d = nc.dram_tensor("xqr", [C, LQ], F32, kind="ExternalInput")
    ident_d = nc.dram_tensor("ident", [128, 128], F32, kind="ExternalInput")
    y_d = nc.dram_tensor("y", [C, LQ], F32, kind="ExternalOutput")

    with tile.TileContext(nc) as tc:
        with tc.tile_pool(name="persist", bufs=1) as pp:
            kt_t = pp.tile([128, NCH, L], BF16)
            qt_t = pp.tile([128, NCH, LQ], BF16)
            vaug_t = pp.tile([128, NL, H * 65], BF16)
            ot_t = pp.tile([128, NCH, LQ], BF16)
            wo_t = pp.tile([128, NCH, C], BF16)
            xqr_t = pp.tile([128, NCH, LQ], F32)
            id_t = pp.tile([128, 128], F32)
            nc.sync.dma_start(out=id_t, in_=ident_d.ap())
            nc.vector.memset(
                vaug_t.rearrange("p lc (h u) -> p lc h u", u=65)[:, :, :, 64], 1.0
            )

            with tc.tile_pool(name="proj_sb", bufs=1) as xp:
                xt = xp.tile([128, NCH, L], BF16)
                wq_t = xp.tile([128, NCH, C], BF16)
                wk_t = xp.tile([128, NCH, C], BF16)
                wv_t = xp.tile([128, NCH, C], BF16)
                # x is the critical early load: give it two DMA queues
                for kc in range(NCH):
                    eng = (nc.sync, nc.scalar)[kc % 2]
                    eng.dma_start(
                        out=xt[:, kc, :],
                        in_=xb_d.ap().rearrange("(c p) l -> p c l", p=128)[:, kc, :],
                    )
                    nc.gpsimd.dma_start(out=wk_t[:, kc, :], in_=wkT_d.ap()[:, kc, :])
                for kc in range(NCH):
                    nc.gpsimd.dma_start(out=wq_t[:, kc, :], in_=wqT_d.ap()[:, kc, :])
                for kc in range(NCH):
                    nc.gpsimd.dma_start(out=wv_t[:, kc, :], in_=wvT_d.ap()[:, kc, :])
                with tc.tile_pool(name="sc_ps", bufs=2, space="PSUM") as scps, \
                     tc.tile_pool(name="ex_sb", bufs=ESB_BUFS) as esb, \
                     tc.tile_pool(name="nrm_sb", bufs=2) as nsb:

                    ex_store = {}

                    def emit_scores(j, lc):
                        # sc_a first: its exp runs on the slower engine (DVE),
                        # so when sc_a's bank frees, sc_b's already has — the
                        # scheduler then compiles a/b matmuls adjacent and the
                        # PE overlaps them (disjoint row groups).
                        sc_a = scps.tile([128, LQ], F32, tag="sc")
                        sc_b = scps.tile([128, LQ], F32, tag="sc")
                        for nq in range(LQ // 512):
                            nc.tensor.matmul(
                                sc_a[:, ts(nq, 512)],
                                kt_t[0:64, j, ts(lc, 128)],
                                qt_t[0:64, j, ts(nq, 512)],
                                start=True,
                                stop=True,
                            )
                            nc.tensor.matmul(
                                sc_b[:, ts(nq, 512)],
                                kt_t[64:128, j, ts(lc, 128)],
                                qt_t[64:128, j, ts(nq, 512)],
                                start=True,
                                stop=True,
                            )
                        ex_a = esb.tile([128, LQ], BF16, tag="ex")
                        # two halves: sc_a frees deterministically AFTER sc_b
                        # (2x760 DVE > 1171 ACT) without serializing the next
                        # chunk's scores behind a monolithic 1.5us DVE op
                        for h in range(2):
                            nc.vector.tensor_scalar(
                                ex_a[:, :].bitcast(I16)[:, ts(h, 512)],
                                sc_a[:, ts(h, 512)],
                                SCHRAU_A,
                                SCHRAU_B,
                                MULT,
                                ADD,
                            )
                        ex_b = esb.tile([128, LQ], BF16, tag="ex")
                        nc.scalar.activation(ex_b[:], sc_b[:], EXP, scale=SCALE)
                        ex_store[(j, lc)] = (ex_a, ex_b)

                    # ------------ phase B+C: V-proj || scores/exp j0 ||
                    # K/Q proj mc=1..3 (kq2 groups spread across the lc loop;
                    # K evicts on DVE, Q evicts on scalar to balance queues)
                    vsc = vaug_t.rearrange("p lc (h u) -> p lc h u", u=65)
                    # mc=0's groups lead (j0 scores need them); the rest spread
                    # through the lc loop
                    kq2_groups = [(0, 0, False), (0, 0, True), (0, 1, True),
                                  (0, 1, False), (0, 2, False), (0, 3, False)]
                    kq2_groups += [(mc, grp, False) for mc in (1, 2, 3) for grp in range(4)]
                    kq2_groups += [(mc, grp, True) for mc in (1, 2, 3) for grp in range(2)]
                    with tc.tile_pool(name="kq2_ps", bufs=2, space="PSUM") as kq2, \
                         tc.tile_pool(name="vp_ps", bufs=2, space="PSUM") as vps:

                        def emit_kq2(idx):
                            mc, grp, is_q = kq2_groups[idx]
                            w_t, dst = (wq_t, qt_t) if is_q else (wk_t, kt_t)
                            ps = kq2.tile([128, 512], F32, tag="kq2", name=f"kq2_{idx}")
                            for kc in range(NCH):
                                nc.tensor.matmul(
                                    ps[:, :],
                                    w_t[:, kc, ts(mc, 128)],
                                    xt[:, kc, ts(grp, 512)],
                                    start=(kc == 0),
                                    stop=(kc == NCH - 1),
                                )
                            if is_q:
                                nc.scalar.copy(dst[:, mc, ts(grp, 512)], ps[:, :])
                            else:
                                nc.vector.tensor_copy(dst[:, mc, ts(grp, 512)], ps[:, :])

                        # phase A equivalent: mc=0 groups up front
                        for gidx in range(6):
                            emit_kq2(gidx)
                        gidx = 6
                        for lc in range(NL):
                            ps = vps.tile([128, 512], F32, tag="vp")
                            for kc in range(NCH):
                                nc.tensor.matmul(
                                    ps[:, :],
                                    xt[:, kc, ts(lc, 128)],
                                    wv_t[:, kc, :],
                                    start=(kc == 0),
                                    stop=(kc == NCH - 1),
                                )
                            # bv is all-zero per the problem spec: plain copy
                            nc.scalar.copy(
                                vsc[:, lc, :, 0:64],
                                ps[:, :].rearrange("p (h u) -> p h u", u=64),
                            )
                            emit_scores(0, lc)
                            while gidx < 6 + (lc + 1) * (len(kq2_groups) - 6) // NL:
                                emit_kq2(gidx)
                                gidx += 1
                        while gidx < len(kq2_groups):
                            emit_kq2(gidx)
                            gidx += 1

                    # wo/xqr are needed only by the out-proj epilogue; load
                    # them after the hot x/weight DMAs so they don't steal
                    # HBM bandwidth from the phase A/B ramp
                    nc.scalar.dma_start(out=wo_t, in_=woT_d.ap())
                    nc.scalar.dma_start(
                        out=xqr_t, in_=xqr_d.ap().rearrange("(c p) l -> p c l", p=128)
                    )

                    # ------------ phase D: attention ------------
                    with tc.tile_pool(name="ut_ps", bufs=2, space="PSUM") as utps:

                        def emit_evict(j, ut_a, ut_b):
                            # scalar eviction frees the UT PSUM banks ~1us
                            # after the last AV; the rest of the normalize
                            # (recip/broadcast/mul) runs off the critical path
                            uts_pair = []
                            for ut in (ut_a, ut_b):
                                den1 = nsb.tile([1, LQ], F32, tag="d1")
                                nc.scalar.copy(den1[:, :], ut[64:65, :])
                                uts = nsb.tile([64, LQ], F32, tag="uts")
                                nc.scalar.copy(uts[:, :], ut[0:64, :])
                                uts_pair.append((uts, den1))
                            return uts_pair

                        def emit_normalize(j, uts_pair):
                            for hi, (uts, den1) in enumerate(uts_pair):
                                inv1 = nsb.tile([1, LQ], F32, tag="i1")
                                nc.vector.reciprocal_approx_fast(
                                    inv1[:, :], den1[:, :]
                                )
                                invb = nsb.tile([64, LQ], F32, tag="invb")
                                nc.gpsimd.partition_broadcast(invb[:, :], inv1[:, :])
                                nc.vector.tensor_mul(
                                    ot_t[64 * hi : 64 * hi + 64, j, :],
                                    uts[:, :],
                                    invb[:, :],
                                )

                        pending = None  # (j, uts_pair) awaiting normalize
                        for j in range(H // 2):
                            ut_a = utps.tile([128, LQ], F32, tag="ut", name=f"uta{j}")
                            ut_b = utps.tile([128, LQ], F32, tag="ut", name=f"utb{j}")

                            def emit_av(j, pl, ut_a=ut_a, ut_b=ut_b):
                                ex_a, ex_b = ex_store[(j, pl)]
                                for hh, ut, ex in (
                                    (2 * j, ut_a, ex_a),
                                    (2 * j + 1, ut_b, ex_b),
                                ):
                                    va = vaug_t[:, pl, ds(hh * 65, 65)]
                                    for nq in range(LQ // 512):
                                        nc.tensor.matmul(
                                            ut[0:65, ts(nq, 512)],
                                            va,
                                            ex[:, ts(nq, 512)],
                                            start=(pl == 0),
                                            stop=(pl == NL - 1),
                                        )

                            if j == 0:
                                # hoist j1's first scores so their exps land
                                # during the AV burst and j1 starts hot
                                for lc in range(3):
                                    emit_scores(1, lc)
                                # scores/exp j0 ran in phase B; drain backlog
                                for pl in range(NL):
                                    emit_av(0, pl)
                            else:
                                for lc in range(NL + 1):
                                    if lc < NL and not (j == 1 and lc < 3):
                                        emit_scores(j, lc)
                                    if lc == 3 and pending is not None:
                                        # normalize of j-1, emitted after the
                                        # first few scores so the DVE queue
                                        # feeds the tensor engine first
                                        emit_normalize(*pending)
                                        pending = None
                                    if lc > 0:
                                        emit_av(j, lc - 1)
                            pending = (j, emit_evict(j, ut_a, ut_b))
                        emit_normalize(*pending)

            # ---------------- out projection + residual ----------------
            with tc.tile_pool(name="op_ps", bufs=1, space="PSUM") as opps, \
                 tc.tile_pool(name="y_sb", bufs=2) as ysb:
                pss = [
                    opps.tile([128, 2, 512], F32, tag=f"op{mc}", name=f"op{mc}")
                    for mc in range(NCH)
                ]
                # residual preloaded into PSUM by an fp32 identity matmul at
                # the START of each accumulation group — runs early, so the
                # tail is just copies + DMA, no serial DVE adds
                for mc in range(NCH):
                    for nq in range(LQ // 512):
                        nc.tensor.matmul(
                            pss[mc][:, nq, :],
                            id_t[:, :],
                            xqr_t[:, mc, ts(nq, 512)],
                            start=True,
                            stop=False,
                            skip_group_check=True,
                        )
                for kc in range(NCH):
                    for mc in range(NCH):
                        for nq in range(LQ // 512):
                            nc.tensor.matmul(
                                pss[mc][:, nq, :],
                                wo_t[:, kc, ts(mc, 128)],
                                ot_t[:, kc, ts(nq, 512)],
                                start=False,
                                stop=(kc == NCH - 1),
                                skip_group_check=True,
                            )
                for mc in range(NCH):
                    y_t = ysb.tile([128, LQ], F32, tag="y")
                    cp = (nc.scalar.copy, nc.vector.tensor_copy)[mc % 2]
                    cp(y_t[:, :], pss[mc].rearrange("p a b -> p (a b)"))
                    eng = (nc.sync, nc.gpsimd, nc.scalar, nc.sync)[mc]
                    eng.dma_start(
                        out=y_d.ap().rearrange("(c p) l -> p c l", p=128)[:, mc, :],
                        in_=y_t,
                    )

    nc.compile()
    return nc


_NC_CACHE = {}


def _get_nc():
    if "nc" not in _NC_CACHE:
        _NC_CACHE["nc"] = build_nc()
    return _NC_CACHE["nc"]


def kernel(x, Wq, bq, Wk, bk, Wv, bv, Wo, bo, _trace=False, _tmpdir=None):
    import ml_dtypes

    npp = ml_dtypes.bfloat16
    x = np.asarray(x, dtype=np.float32)
    assert np.abs(np.asarray(bq)).max() == 0.0
    assert np.abs(np.asarray(bk)).max() == 0.0
    assert np.abs(np.asarray(bv)).max() == 0.0
    assert np.abs(np.asarray(bo)).max() == 0.0
    nc = _get_nc()

    def _tile_w(w):
        wT = np.asarray(w, np.float32).T.reshape(NCH, 128, C).transpose(1, 0, 2)
        return np.ascontiguousarray(wT).astype(npp)

    shared = {
        "wqT": _tile_w(Wq),
        "wkT": _tile_w(Wk),
        "wvT": _tile_w(Wv),
        "woT": _tile_w(Wo),
        "ident": np.eye(128, dtype=np.float32),
    }
    in_maps = []
    for core in range(NCORES):
        b, half = core // 2, core % 2
        xb = x[b]
        # rotate so this core's query half occupies columns 0:LQ; attention
        # is invariant to key order, and all other uses are column-sliced
        xrot = np.ascontiguousarray(
            np.concatenate(
                [
                    xb[:, half * LQ : (half + 1) * LQ],
                    xb[:, (1 - half) * LQ : (2 - half) * LQ],
                ],
                axis=1,
            )
        )
        m = dict(shared)
        m["xb"] = xrot.astype(npp)
        m["xqr"] = np.ascontiguousarray(xrot[:, 0:LQ])
        in_maps.append(m)

    res = run_bass_kernel_spmd(
        nc, in_maps, list(range(NCORES)), trace=_trace, tmpdir=_tmpdir
    )

    y = np.empty((B, C, L), np.float32)
    for core in range(NCORES):
        b, half = core // 2, core % 2
        y[b, :, half * LQ : (half + 1) * LQ] = res.results[core]["y"]
    kernel.last_exec_time_ns = res.exec_time_ns if _trace else None
    return y


# revision 36
# speedup vs baseline: 1.0447x; 1.0447x over previous
"""MobileMQA1D attention block on 8 Trainium2 NeuronCores.

Reference computation (B=4, C=512, L=2048, H=8, D=64):
    xp = x.T                     # (L, C) per batch
    q/k/v = xp @ W.T + b         # heads (H, L, D)
    attn  = softmax(q k^T / sqrt(D))
    out   = (attn @ v) reassembled -> @ Wo.T + bo
    y     = x + out.T            # (C, L) per batch

Sharding: 8 cores = 4 batches x 2 query-halves. Each core computes K/V
for its whole batch and Q/attention/out-proj for its 1024-query half.
No cross-core communication.

Channel-first layout ("transposed scores") so the softmax sum rides the
matmul contraction axis (augmented V row 64 = ones -> denominator).

Engine split per head pair (keeps all three engines off the serial
path): head a's exp is a Schraudolph bf16-bitcast approximation on
VectorE (one tensor_scalar: i16 = s*scale*2^7/ln2 + 127*2^7 - C);
head b's exp is exact on ScalarE. sc_a frees LATER than sc_b (DVE 1454
vs ACT 1114 ns), so emitting sc_a's matmuls first makes the scheduler
see both score tiles co-ready and compile the h0/h64 matmuls adjacent,
where the PE runs them concurrently in distinct row groups.

PSUM (8 banks): A: kq1 4+2 | B: vp 2 + sc 4 | C: kq2 2 + sc 4
              | D: sc 4 + ut 4 | tail: op 8
"""

import math
import sys

sys.path.insert(0, "/opt/trn_rl_repo")


import numpy as np

import concourse.bass as bass
import concourse.mybir as mybir
import concourse.tile as tile
from concourse import bacc
from concourse.bass import ds, ts
from concourse.bass_utils import run_bass_kernel_spmd

F32 = mybir.dt.float32
BF16 = mybir.dt.bfloat16
I16 = mybir.dt.int16
EXP = mybir.ActivationFunctionType.Exp
MULT = mybir.AluOpType.mult
ADD = mybir.AluOpType.add

B, C, L, H = 4, 512, 2048, 8
D = C // H
LQ = L // 2
SCALE = float(D) ** -0.5
NCORES = 8
NL = L // 128  # 16 key chunks
NCH = C // 128  # 4 channel chunks

ESB_BUFS = 38  # j0's 32 exp tiles buffered through phase B + j1 hoist + slack

# Schraudolph exp in bf16 bit patterns (HW rounds to nearest)
SCHRAU_A = float(SCALE * (2.0**7) / math.log(2.0))
SCHRAU_B = float(127.0 * 2.0**7 - 0.0438 * 2.0**7)


def build_nc():
    nc = bacc.Bacc("TRN2", target_bir_lowering=False, debug=False)

    xb_d = nc.dram_tensor("xb", [C, L], BF16, kind="ExternalInput")
    wqT_d = nc.dram_tensor("wqT", [128, NCH, C], BF16, kind="ExternalInput")
    wkT_d = nc.dram_tensor("wkT", [128, NCH, C], BF16, kind="ExternalInput")
    wvT_d = nc.dram_tensor("wvT", [128, NCH, C], BF16, kind="ExternalInput")
    woT_d = nc.dram_tensor("woT", [128, NCH, C], BF16, kind="ExternalInput")
    xqr_d = nc.dram_tensor("xqr", [C, LQ], F32, kind="ExternalInput")
    ident_d = nc.dram_tensor("ident", [128, 128], F32, kind="ExternalInput")
    y_d = nc.dram_tensor("y", [C, LQ], F32, kind="ExternalOutput")

    with tile.TileContext(nc) as tc:
        with tc.tile_pool(name="persist", bufs=1) as pp:
            kt_t = pp.tile([128, NCH, L], BF16)
            qt_t = pp.tile([128, NCH, LQ], BF16)
            vaug_t = pp.tile([128, NL, H * 65], BF16)
            ot_t = pp.tile([128, NCH, LQ], BF16)
            wo_t = pp.tile([128, NCH, C], BF16)
            xqr_t = pp.tile([128, NCH, LQ], F32)
            id_t = pp.tile([128, 128], F32)
            nc.sync.dma_start(out=id_t, in_=ident_d.ap())
            nc.vector.memset(
                vaug_t.rearrange("p lc (h u) -> p lc h u", u=65)[:, :, :, 64], 1.0
            )

            with tc.tile_pool(name="proj_sb", bufs=1) as xp:
                xt = xp.tile([128, NCH, L], BF16)
                wq_t = xp.tile([128, NCH, C], BF16)
                wk_t = xp.tile([128, NCH, C], BF16)
                wv_t = xp.tile([128, NCH, C], BF16)
                # x is the critical early load: give it two DMA queues
                for kc in range(NCH):
                    eng = (nc.sync, nc.scalar)[kc % 2]
                    eng.dma_start(
                        out=xt[:, kc, :],
                        in_=xb_d.ap().rearrange("(c p) l -> p c l", p=128)[:, kc, :],
                    )
                    nc.gpsimd.dma_start(out=wk_t[:, kc, :], in_=wkT_d.ap()[:, kc, :])
                for kc in range(NCH):
                    nc.gpsimd.dma_start(out=wq_t[:, kc, :], in_=wqT_d.ap()[:, kc, :])
                for kc in range(NCH):
                    nc.gpsimd.dma_start(out=wv_t[:, kc, :], in_=wvT_d.ap()[:, kc, :])
                with tc.tile_pool(name="sc_ps", bufs=2, space="PSUM") as scps, \
                     tc.tile_pool(name="ex_sb", bufs=ESB_BUFS) as esb, \
                     tc.tile_pool(name="nrm_sb", bufs=2) as nsb:

                    ex_store = {}

                    def emit_scores(j, lc):
                        # sc_a first: its exp runs on the slower engine (DVE),
                        # so when sc_a's bank frees, sc_b's already has — the
                        # scheduler then compiles a/b matmuls adjacent and the
                        # PE overlaps them (disjoint row groups).
                        sc_a = scps.tile([128, LQ], F32, tag="sc")
                        sc_b = scps.tile([128, LQ], F32, tag="sc")
                        for nq in range(LQ // 512):
                            nc.tensor.matmul(
                                sc_a[:, ts(nq, 512)],
                                kt_t[0:64, j, ts(lc, 128)],
                                qt_t[0:64, j, ts(nq, 512)],
                                start=True,
                                stop=True,
                            )
                            nc.tensor.matmul(
                                sc_b[:, ts(nq, 512)],
                                kt_t[64:128, j, ts(lc, 128)],
                                qt_t[64:128, j, ts(nq, 512)],
                                start=True,
                                stop=True,
                            )
                        ex_a = esb.tile([128, LQ], BF16, tag="ex")
                        # two halves: sc_a frees deterministically AFTER sc_b
                        # (2x760 DVE > 1171 ACT) without serializing the next
                        # chunk's scores behind a monolithic 1.5us DVE op
                        for h in range(2):
                            nc.vector.tensor_scalar(
                                ex_a[:, :].bitcast(I16)[:, ts(h, 512)],
                                sc_a[:, ts(h, 512)],
                                SCHRAU_A,
                                SCHRAU_B,
                                MULT,
                                ADD,
                            )
                        ex_b = esb.tile([128, LQ], BF16, tag="ex")
                        nc.scalar.activation(ex_b[:], sc_b[:], EXP, scale=SCALE)
                        ex_store[(j, lc)] = (ex_a, ex_b)

                    # ------------ phase B+C: V-proj || scores/exp j0 ||
                    # K/Q proj mc=1..3 (kq2 groups spread across the lc loop;
                    # K evicts on DVE, Q evicts on scalar to balance queues)
                    vsc = vaug_t.rearrange("p lc (h u) -> p lc h u", u=65)
                    # mc=0's groups lead (j0 scores need them); the rest spread
                    # through the lc loop
                    kq2_groups = [(0, 0, False), (0, 0, True), (0, 1, True),
                                  (0, 1, False), (0, 2, False), (0, 3, False)]
                    kq2_groups += [(mc, grp, False) for mc in (1, 2, 3) for grp in range(4)]
                    kq2_groups += [(mc, grp, True) for mc in (1, 2, 3) for grp in range(2)]
                    with tc.tile_pool(name="kq2_ps", bufs=2, space="PSUM") as kq2, \
                         tc.tile_pool(name="vp_ps", bufs=2, space="PSUM") as vps:

                        def emit_kq2(idx):
                            mc, grp, is_q = kq2_groups[idx]
                            w_t, dst = (wq_t, qt_t) if is_q else (wk_t, kt_t)
                            ps = kq2.tile([128, 512], F32, tag="kq2", name=f"kq2_{idx}")
                            for kc in range(NCH):
                                nc.tensor.matmul(
                                    ps[:, :],
                                    w_t[:, kc, ts(mc, 128)],
                                    xt[:, kc, ts(grp, 512)],
                                    start=(kc == 0),
                                    stop=(kc == NCH - 1),
                                )
                            if is_q:
                                nc.scalar.copy(dst[:, mc, ts(grp, 512)], ps[:, :])
                            else:
                                nc.vector.tensor_copy(dst[:, mc, ts(grp, 512)], ps[:, :])

                        # phase A equivalent: mc=0 groups up front
                        for gidx in range(6):
                            emit_kq2(gidx)
                        gidx = 6
                        for lc in range(NL):
                            ps = vps.tile([128, 512], F32, tag="vp")
                            for kc in range(NCH):
                                nc.tensor.matmul(
                                    ps[:, :],
                                    xt[:, kc, ts(lc, 128)],
                                    wv_t[:, kc, :],
                                    start=(kc == 0),
                                    stop=(kc == NCH - 1),
                                )
                            # bv is all-zero per the problem spec: plain copy
                            nc.scalar.copy(
                                vsc[:, lc, :, 0:64],
                                ps[:, :].rearrange("p (h u) -> p h u", u=64),
                            )
                            emit_scores(0, lc)
                            while gidx < 6 + (lc + 1) * (len(kq2_groups) - 6) // NL:
                                emit_kq2(gidx)
                                gidx += 1
                        while gidx < len(kq2_groups):
                            emit_kq2(gidx)
                            gidx += 1

                    # wo/xqr are needed only by the out-proj epilogue; load
                    # them after the hot x/weight DMAs so they don't steal
                    # HBM bandwidth from the phase A/B ramp
                    nc.scalar.dma_start(out=wo_t, in_=woT_d.ap())
                    nc.scalar.dma_start(
                        out=xqr_t, in_=xqr_d.ap().rearrange("(c p) l -> p c l", p=128)
                    )

                    # ------------ phase D: attention ------------
                    with tc.tile_pool(name="ut_ps", bufs=2, space="PSUM") as utps:

                        def emit_evict(j, ut_a, ut_b):
                            # scalar eviction frees the UT PSUM banks ~1us
                            # after the last AV; the rest of the normalize
                            # (recip/broadcast/mul) runs off the critical path
                            uts_pair = []
                            for ut in (ut_a, ut_b):
                                den1 = nsb.tile([1, LQ], F32, tag="d1")
                                nc.scalar.copy(den1[:, :], ut[64:65, :])
                                uts = nsb.tile([64, LQ], F32, tag="uts")
                                nc.scalar.copy(uts[:, :], ut[0:64, :])
                                uts_pair.append((uts, den1))
                            return uts_pair

                        def emit_normalize(j, uts_pair):
                            for hi, (uts, den1) in enumerate(uts_pair):
                                inv1 = nsb.tile([1, LQ], F32, tag="i1")
                                nc.vector.reciprocal_approx_fast(
                                    inv1[:, :], den1[:, :]
                                )
                                invb = nsb.tile([64, LQ], F32, tag="invb")
                                nc.gpsimd.partition_broadcast(invb[:, :], inv1[:, :])
                                nc.vector.tensor_mul(
                                    ot_t[64 * hi : 64 * hi + 64, j, :],
                                    uts[:, :],
                                    invb[:, :],
                                )

                        pending = None  # (j, uts_pair) awaiting normalize
                        for j in range(H // 2):
                            ut_a = utps.tile([128, LQ], F32, tag="ut", name=f"uta{j}")
                            ut_b = utps.tile([128, LQ], F32, tag="ut", name=f"utb{j}")

                            def emit_av(j, pl, ut_a=ut_a, ut_b=ut_b):
                                ex_a, ex_b = ex_store[(j, pl)]
                                for hh, ut, ex in (
                                    (2 * j, ut_a, ex_a),
                                    (2 * j + 1, ut_b, ex_b),
                                ):
                                    va = vaug_t[:, pl, ds(hh * 65, 65)]
                                    for nq in range(LQ // 512):
                                        nc.tensor.matmul(
                                            ut[0:65, ts(nq, 512)],
                                            va,
                                            ex[:, ts(nq, 512)],
                                            start=(pl == 0),
                                            stop=(pl == NL - 1),
                                        )

                            if j == 0:
                                # hoist j1's first scores so their exps land
                                # during the AV burst and j1 starts hot
                                for lc in range(3):
                                    emit_scores(1, lc)
                                # scores/exp j0 ran in phase B; drain backlog
                                for pl in range(NL):
                                    emit_av(0, pl)
                            else:
                                for lc in range(NL + 1):
                                    if lc < NL and not (j == 1 and lc < 3):
                                        emit_scores(j, lc)
                                    if lc == 3 and pending is not None:
                                        # normalize of j-1, emitted after the
                                        # first few scores so the DVE queue
                                        # feeds the tensor engine first
                                        emit_normalize(*pending)
                                        pending = None
                                    if lc > 0:
                                        emit_av(j, lc - 1)
                            pending = (j, emit_evict(j, ut_a, ut_b))
                        emit_normalize(*pending)

            # ---------------- out projection + residual ----------------
            with tc.tile_pool(name="op_ps", bufs=1, space="PSUM") as opps, \
                 tc.tile_pool(name="y_sb", bufs=2) as ysb:
                pss = [
                    opps.tile([128, 2, 512], F32, tag=f"op{mc}", name=f"op{mc}")
                    for mc in range(NCH)
                ]
                for kc in range(NCH):
                    for mc in range(NCH):
                        for nq in range(LQ // 512):
                            nc.tensor.matmul(
                                pss[mc][:, nq, :],
                                wo_t[:, kc, ts(mc, 128)],
                                ot_t[:, kc, ts(nq, 512)],
                                start=(kc == 0),
                                stop=(kc == NCH - 1),
                            )
                for mc in range(NCH):
                    y_t = ysb.tile([128, LQ], F32, tag="y")
                    nc.vector.tensor_add(
                        y_t[:, :],
                        pss[mc].rearrange("p a b -> p (a b)"),
                        xqr_t[:, mc, :],
                    )
                    eng = (nc.sync, nc.gpsimd, nc.scalar, nc.sync)[mc]
                    eng.dma_start(
                        out=y_d.ap().rearrange("(c p) l -> p c l", p=128)[:, mc, :],
                        in_=y_t,
                    )

    nc.compile()
    return nc


_NC_CACHE = {}


def _get_nc():
    if "nc" not in _NC_CACHE:
        _NC_CACHE["nc"] = build_nc()
    return _NC_CACHE["nc"]


def kernel(x, Wq, bq, Wk, bk, Wv, bv, Wo, bo, _trace=False, _tmpdir=None):
    import ml_dtypes

    npp = ml_dtypes.bfloat16
    x = np.asarray(x, dtype=np.float32)
    assert np.abs(np.asarray(bq)).max() == 0.0
    assert np.abs(np.asarray(bk)).max() == 0.0
    assert np.abs(np.asarray(bv)).max() == 0.0
    assert np.abs(np.asarray(bo)).max() == 0.0
    nc = _get_nc()

    def _tile_w(w):
        wT = np.asarray(w, np.float32).T.reshape(NCH, 128, C).transpose(1, 0, 2)
        return np.ascontiguousarray(wT).astype(npp)

    shared = {
        "wqT": _tile_w(Wq),
        "wkT": _tile_w(Wk),
        "wvT": _tile_w(Wv),
        "woT": _tile_w(Wo),
        "ident": np.eye(128, dtype=np.float32),
    }
    in_maps = []
    for core in range(NCORES):
        b, half = core // 2, core % 2
        xb = x[b]
        # rotate so this core's query half occupies columns 0:LQ; attention
        # is invariant to key order, and all other uses are column-sliced
        xrot = np.ascontiguousarray(
            np.concatenate(
                [
                    xb[:, half * LQ : (half + 1) * LQ],
                    xb[:, (1 - half) * LQ : (2 - half) * LQ],
                ],
                axis=1,
            )
        )
        m = dict(shared)
        m["xb"] = xrot.astype(npp)
        m["xqr"] = np.ascontiguousarray(xrot[:, 0:LQ])
        in_maps.append(m)

    res = run_bass_kernel_spmd(
        nc, in_maps, list(range(NCORES)), trace=_trace, tmpdir=_tmpdir
    )

    y = np.empty((B, C, L), np.float32)
    for core in range(NCORES):
        b, half = core // 2, core % 2
        y[b, :, half * LQ : (half + 1) * LQ] = res.results[core]["y"]
    kernel.last_exec_time_ns = res.exec_time_ns if _trace else None
    return y


# revision 37
# speedup vs baseline: 1.0584x; 1.0131x over previous
"""MobileMQA1D attention block on 8 Trainium2 NeuronCores.

Reference computation (B=4, C=512, L=2048, H=8, D=64):
    xp = x.T                     # (L, C) per batch
    q/k/v = xp @ W.T + b         # heads (H, L, D)
    attn  = softmax(q k^T / sqrt(D))
    out   = (attn @ v) reassembled -> @ Wo.T + bo
    y     = x + out.T            # (C, L) per batch

Sharding: 8 cores = 4 batches x 2 query-halves. Each core computes K/V
for its whole batch and Q/attention/out-proj for its 1024-query half.
No cross-core communication.

Channel-first layout ("transposed scores") so the softmax sum rides the
matmul contraction axis (augmented V row 64 = ones -> denominator).

Engine split per head pair (keeps all three engines off the serial
path): head a's exp is a Schraudolph bf16-bitcast approximation on
VectorE (one tensor_scalar: i16 = s*scale*2^7/ln2 + 127*2^7 - C);
head b's exp is exact on ScalarE. sc_a frees LATER than sc_b (DVE 1454
vs ACT 1114 ns), so emitting sc_a's matmuls first makes the scheduler
see both score tiles co-ready and compile the h0/h64 matmuls adjacent,
where the PE runs them concurrently in distinct row groups.

PSUM (8 banks): A: kq1 4+2 | B: vp 2 + sc 4 | C: kq2 2 + sc 4
              | D: sc 4 + ut 4 | tail: op 8
"""

import math
import sys

sys.path.insert(0, "/opt/trn_rl_repo")


import numpy as np

import concourse.bass as bass
import concourse.mybir as mybir
import concourse.tile as tile
from concourse import bacc
from concourse.bass import ds, ts
from concourse.bass_utils import run_bass_kernel_spmd

F32 = mybir.dt.float32
BF16 = mybir.dt.bfloat16
I16 = mybir.dt.int16
EXP = mybir.ActivationFunctionType.Exp
MULT = mybir.AluOpType.mult
ADD = mybir.AluOpType.add

B, C, L, H = 4, 512, 2048, 8
D = C // H
LQ = L // 2
SCALE = float(D) ** -0.5
NCORES = 8
NL = L // 128  # 16 key chunks
NCH = C // 128  # 4 channel chunks

ESB_BUFS = 38  # j0's 32 exp tiles buffered through phase B + j1 hoist + slack

# Schraudolph exp in bf16 bit patterns (HW rounds to nearest)
SCHRAU_A = float(SCALE * (2.0**7) / math.log(2.0))
SCHRAU_B = float(127.0 * 2.0**7 - 0.0438 * 2.0**7)


def build_nc():
    nc = bacc.Bacc("TRN2", target_bir_lowering=False, debug=False)

    xb_d = nc.dram_tensor("xb", [C, L], BF16, kind="ExternalInput")
    wqT_d = nc.dram_tensor("wqT", [128, NCH, C], BF16, kind="ExternalInput")
    wkT_d = nc.dram_tensor("wkT", [128, NCH, C], BF16, kind="ExternalInput")
    wvT_d = nc.dram_tensor("wvT", [128, NCH, C], BF16, kind="ExternalInput")
    woT_d = nc.dram_tensor("woT", [128, NCH, C], BF16, kind="ExternalInput")
    xqr_d = nc.dram_tensor("xqr", [C, LQ], F32, kind="ExternalInput")
    ident_d = nc.dram_tensor("ident", [128, 128], F32, kind="ExternalInput")
    y_d = nc.dram_tensor("y", [C, LQ], F32, kind="ExternalOutput")

    with tile.TileContext(nc) as tc:
        with tc.tile_pool(name="persist", bufs=1) as pp:
            kt_t = pp.tile([128, NCH, L], BF16)
            qt_t = pp.tile([128, NCH, LQ], BF16)
            vaug_t = pp.tile([128, NL, H * 65], BF16)
            ot_t = pp.tile([128, NCH, LQ], BF16)
            wo_t = pp.tile([128, NCH, C], BF16)
            xqr_t = pp.tile([128, NCH, LQ], F32)
            id_t = pp.tile([128, 128], F32)
            nc.sync.dma_start(out=id_t, in_=ident_d.ap())
            nc.vector.memset(
                vaug_t.rearrange("p lc (h u) -> p lc h u", u=65)[:, :, :, 64], 1.0
            )

            with tc.tile_pool(name="proj_sb", bufs=1) as xp:
                xt = xp.tile([128, NCH, L], BF16)
                wq_t = xp.tile([128, NCH, C], BF16)
                wk_t = xp.tile([128, NCH, C], BF16)
                wv_t = xp.tile([128, NCH, C], BF16)
                # x is the critical early load: give it two DMA queues; wv
                # rides behind x on those queues (V-proj needs all 4 chunks)
                # while wk/wq go on gpsimd
                for kc in range(NCH):
                    eng = (nc.sync, nc.scalar)[kc % 2]
                    eng.dma_start(
                        out=xt[:, kc, :],
                        in_=xb_d.ap().rearrange("(c p) l -> p c l", p=128)[:, kc, :],
                    )
                    nc.gpsimd.dma_start(out=wk_t[:, kc, :], in_=wkT_d.ap()[:, kc, :])
                for kc in range(NCH):
                    nc.gpsimd.dma_start(out=wq_t[:, kc, :], in_=wqT_d.ap()[:, kc, :])
                for kc in range(NCH):
                    eng = (nc.sync, nc.scalar)[kc % 2]
                    eng.dma_start(out=wv_t[:, kc, :], in_=wvT_d.ap()[:, kc, :])
                with tc.tile_pool(name="sc_ps", bufs=2, space="PSUM") as scps, \
                     tc.tile_pool(name="ex_sb", bufs=ESB_BUFS) as esb, \
                     tc.tile_pool(name="nrm_sb", bufs=2) as nsb:

                    ex_store = {}

                    def emit_scores(j, lc):
                        # sc_a first: its exp runs on the slower engine (DVE),
                        # so when sc_a's bank frees, sc_b's already has — the
                        # scheduler then compiles a/b matmuls adjacent and the
                        # PE overlaps them (disjoint row groups).
                        sc_a = scps.tile([128, LQ], F32, tag="sc")
                        sc_b = scps.tile([128, LQ], F32, tag="sc")
                        for nq in range(LQ // 512):
                            nc.tensor.matmul(
                                sc_a[:, ts(nq, 512)],
                                kt_t[0:64, j, ts(lc, 128)],
                                qt_t[0:64, j, ts(nq, 512)],
                                start=True,
                                stop=True,
                            )
                            nc.tensor.matmul(
                                sc_b[:, ts(nq, 512)],
                                kt_t[64:128, j, ts(lc, 128)],
                                qt_t[64:128, j, ts(nq, 512)],
                                start=True,
                                stop=True,
                            )
                        ex_a = esb.tile([128, LQ], BF16, tag="ex")
                        # two halves: sc_a frees deterministically AFTER sc_b
                        # (2x760 DVE > 1171 ACT) without serializing the next
                        # chunk's scores behind a monolithic 1.5us DVE op
                        for h in range(2):
                            nc.vector.tensor_scalar(
                                ex_a[:, :].bitcast(I16)[:, ts(h, 512)],
                                sc_a[:, ts(h, 512)],
                                SCHRAU_A,
                                SCHRAU_B,
                                MULT,
                                ADD,
                            )
                        ex_b = esb.tile([128, LQ], BF16, tag="ex")
                        nc.scalar.activation(ex_b[:], sc_b[:], EXP, scale=SCALE)
                        ex_store[(j, lc)] = (ex_a, ex_b)

                    # ------------ phase B+C: V-proj || scores/exp j0 ||
                    # K/Q proj mc=1..3 (kq2 groups spread across the lc loop;
                    # K evicts on DVE, Q evicts on scalar to balance queues)
                    vsc = vaug_t.rearrange("p lc (h u) -> p lc h u", u=65)
                    # mc=0's groups lead (j0 scores need them); the rest spread
                    # through the lc loop
                    kq2_groups = [(0, 0, False), (0, 0, True), (0, 1, True),
                                  (0, 1, False), (0, 2, False), (0, 3, False)]
                    kq2_groups += [(mc, grp, False) for mc in (1, 2, 3) for grp in range(4)]
                    kq2_groups += [(mc, grp, True) for mc in (1, 2, 3) for grp in range(2)]
                    with tc.tile_pool(name="kq2_ps", bufs=2, space="PSUM") as kq2, \
                         tc.tile_pool(name="vp_ps", bufs=2, space="PSUM") as vps:

                        def emit_kq2(idx):
                            mc, grp, is_q = kq2_groups[idx]
                            w_t, dst = (wq_t, qt_t) if is_q else (wk_t, kt_t)
                            ps = kq2.tile([128, 512], F32, tag="kq2", name=f"kq2_{idx}")
                            for kc in range(NCH):
                                nc.tensor.matmul(
                                    ps[:, :],
                                    w_t[:, kc, ts(mc, 128)],
                                    xt[:, kc, ts(grp, 512)],
                                    start=(kc == 0),
                                    stop=(kc == NCH - 1),
                                )
                            if is_q:
                                nc.scalar.copy(dst[:, mc, ts(grp, 512)], ps[:, :])
                            else:
                                nc.vector.tensor_copy(dst[:, mc, ts(grp, 512)], ps[:, :])

                        # phase A equivalent: mc=0 groups up front
                        for gidx in range(6):
                            emit_kq2(gidx)
                        gidx = 6
                        for lc in range(NL):
                            ps = vps.tile([128, 512], F32, tag="vp")
                            for kc in range(NCH):
                                nc.tensor.matmul(
                                    ps[:, :],
                                    xt[:, kc, ts(lc, 128)],
                                    wv_t[:, kc, :],
                                    start=(kc == 0),
                                    stop=(kc == NCH - 1),
                                )
                            # bv is all-zero per the problem spec: plain copy
                            nc.scalar.copy(
                                vsc[:, lc, :, 0:64],
                                ps[:, :].rearrange("p (h u) -> p h u", u=64),
                            )
                            emit_scores(0, lc)
                            while gidx < 6 + (lc + 1) * (len(kq2_groups) - 6) // NL:
                                emit_kq2(gidx)
                                gidx += 1
                        while gidx < len(kq2_groups):
                            emit_kq2(gidx)
                            gidx += 1

                    # wo/xqr are needed only by the out-proj epilogue; load
                    # them after the hot x/weight DMAs so they don't steal
                    # HBM bandwidth from the phase A/B ramp
                    nc.scalar.dma_start(out=wo_t, in_=woT_d.ap())
                    nc.scalar.dma_start(
                        out=xqr_t, in_=xqr_d.ap().rearrange("(c p) l -> p c l", p=128)
                    )

                    # ------------ phase D: attention ------------
                    with tc.tile_pool(name="ut_ps", bufs=2, space="PSUM") as utps:

                        def emit_evict(j, ut_a, ut_b):
                            # scalar eviction frees the UT PSUM banks ~1us
                            # after the last AV; the rest of the normalize
                            # (recip/broadcast/mul) runs off the critical path
                            uts_pair = []
                            for ut in (ut_a, ut_b):
                                den1 = nsb.tile([1, LQ], F32, tag="d1")
                                nc.scalar.copy(den1[:, :], ut[64:65, :])
                                uts = nsb.tile([64, LQ], F32, tag="uts")
                                nc.scalar.copy(uts[:, :], ut[0:64, :])
                                uts_pair.append((uts, den1))
                            return uts_pair

                        def emit_normalize(j, uts_pair):
                            for hi, (uts, den1) in enumerate(uts_pair):
                                inv1 = nsb.tile([1, LQ], F32, tag="i1")
                                nc.vector.reciprocal_approx_fast(
                                    inv1[:, :], den1[:, :]
                                )
                                invb = nsb.tile([64, LQ], F32, tag="invb")
                                nc.gpsimd.partition_broadcast(invb[:, :], inv1[:, :])
                                nc.vector.tensor_mul(
                                    ot_t[64 * hi : 64 * hi + 64, j, :],
                                    uts[:, :],
                                    invb[:, :],
                                )

                        pending = None  # (j, uts_pair) awaiting normalize
                        for j in range(H // 2):
                            ut_a = utps.tile([128, LQ], F32, tag="ut", name=f"uta{j}")
                            ut_b = utps.tile([128, LQ], F32, tag="ut", name=f"utb{j}")

                            def emit_av(j, pl, ut_a=ut_a, ut_b=ut_b):
                                ex_a, ex_b = ex_store[(j, pl)]
                                for hh, ut, ex in (
                                    (2 * j, ut_a, ex_a),
                                    (2 * j + 1, ut_b, ex_b),
                                ):
                                    va = vaug_t[:, pl, ds(hh * 65, 65)]
                                    for nq in range(LQ // 512):
                                        nc.tensor.matmul(
                                            ut[0:65, ts(nq, 512)],
                                            va,
                                            ex[:, ts(nq, 512)],
                                            start=(pl == 0),
                                            stop=(pl == NL - 1),
                                        )

                            if j == 0:
                                # hoist j1's first scores so their exps land
                                # during the AV burst and j1 starts hot
                                for lc in range(3):
                                    emit_scores(1, lc)
                                # scores/exp j0 ran in phase B; drain backlog
                                for pl in range(NL):
                                    emit_av(0, pl)
                            else:
                                for lc in range(NL + 1):
                                    if lc < NL and not (j == 1 and lc < 3):
                                        emit_scores(j, lc)
                                    if lc == 3 and pending is not None:
                                        # normalize of j-1, emitted after the
                                        # first few scores so the DVE queue
                                        # feeds the tensor engine first
                                        emit_normalize(*pending)
                                        pending = None
                                    if lc > 0:
                                        emit_av(j, lc - 1)
                            pending = (j, emit_evict(j, ut_a, ut_b))
                        emit_normalize(*pending)

            # ---------------- out projection + residual ----------------
            with tc.tile_pool(name="op_ps", bufs=1, space="PSUM") as opps, \
                 tc.tile_pool(name="y_sb", bufs=2) as ysb:
                pss = [
                    opps.tile([128, 2, 512], F32, tag=f"op{mc}", name=f"op{mc}")
                    for mc in range(NCH)
                ]
                for kc in range(NCH):
                    for mc in range(NCH):
                        for nq in range(LQ // 512):
                            nc.tensor.matmul(
                                pss[mc][:, nq, :],
                                wo_t[:, kc, ts(mc, 128)],
                                ot_t[:, kc, ts(nq, 512)],
                                start=(kc == 0),
                                stop=(kc == NCH - 1),
                            )
                for mc in range(NCH):
                    y_t = ysb.tile([128, LQ], F32, tag="y")
                    nc.vector.tensor_add(
                        y_t[:, :],
                        pss[mc].rearrange("p a b -> p (a b)"),
                        xqr_t[:, mc, :],
                    )
                    eng = (nc.sync, nc.gpsimd, nc.scalar, nc.sync)[mc]
                    eng.dma_start(
                        out=y_d.ap().rearrange("(c p) l -> p c l", p=128)[:, mc, :],
                        in_=y_t,
                    )

    nc.compile()
    return nc


_NC_CACHE = {}


def _get_nc():
    if "nc" not in _NC_CACHE:
        _NC_CACHE["nc"] = build_nc()
    return _NC_CACHE["nc"]


def kernel(x, Wq, bq, Wk, bk, Wv, bv, Wo, bo, _trace=False, _tmpdir=None):
    import ml_dtypes

    npp = ml_dtypes.bfloat16
    x = np.asarray(x, dtype=np.float32)
    assert np.abs(np.asarray(bq)).max() == 0.0
    assert np.abs(np.asarray(bk)).max() == 0.0
    assert np.abs(np.asarray(bv)).max() == 0.0
    assert np.abs(np.asarray(bo)).max() == 0.0
    nc = _get_nc()

    def _tile_w(w):
        wT = np.asarray(w, np.float32).T.reshape(NCH, 128, C).transpose(1, 0, 2)
        return np.ascontiguousarray(wT).astype(npp)

    shared = {
        "wqT": _tile_w(Wq),
        "wkT": _tile_w(Wk),
        "wvT": _tile_w(Wv),
        "woT": _tile_w(Wo),
        "ident": np.eye(128, dtype=np.float32),
    }
    in_maps = []
    for core in range(NCORES):
        b, half = core // 2, core % 2
        xb = x[b]
        # rotate so this core's query half occupies columns 0:LQ; attention
        # is invariant to key order, and all other uses are column-sliced
        xrot = np.ascontiguousarray(
            np.concatenate(
                [
                    xb[:, half * LQ : (half + 1) * LQ],
                    xb[:, (1 - half) * LQ : (2 - half) * LQ],
                ],
                axis=1,
            )
        )
        m = dict(shared)
        m["xb"] = xrot.astype(npp)
        m["xqr"] = np.ascontiguousarray(xrot[:, 0:LQ])
        in_maps.append(m)

    res = run_bass_kernel_spmd(
        nc, in_maps, list(range(NCORES)), trace=_trace, tmpdir=_tmpdir
    )

    y = np.empty((B, C, L), np.float32)
    for core in range(NCORES):
        b, half = core // 2, core % 2
        y[b, :, half * LQ : (half + 1) * LQ] = res.results[core]["y"]
    kernel.last_exec_time_ns = res.exec_time_ns if _trace else None
    return y


# revision 44
# speedup vs baseline: 1.0642x; 1.0054x over previous
"""MobileMQA1D attention block on 8 Trainium2 NeuronCores.

Reference computation (B=4, C=512, L=2048, H=8, D=64):
    xp = x.T                     # (L, C) per batch
    q/k/v = xp @ W.T + b         # heads (H, L, D)
    attn  = softmax(q k^T / sqrt(D))
    out   = (attn @ v) reassembled -> @ Wo.T + bo
    y     = x + out.T            # (C, L) per batch

Sharding: 8 cores = 4 batches x 2 query-halves. Each core computes K/V
for its whole batch and Q/attention/out-proj for its 1024-query half.
No cross-core communication.

Channel-first layout ("transposed scores") so the softmax sum rides the
matmul contraction axis (augmented V row 64 = ones -> denominator).

Engine split per head pair (keeps all three engines off the serial
path): head a's exp is a Schraudolph bf16-bitcast approximation on
VectorE (one tensor_scalar: i16 = s*scale*2^7/ln2 + 127*2^7 - C);
head b's exp is exact on ScalarE. sc_a frees LATER than sc_b (DVE 1454
vs ACT 1114 ns), so emitting sc_a's matmuls first makes the scheduler
see both score tiles co-ready and compile the h0/h64 matmuls adjacent,
where the PE runs them concurrently in distinct row groups.

PSUM (8 banks): A: kq1 4+2 | B: vp 2 + sc 4 | C: kq2 2 + sc 4
              | D: sc 4 + ut 4 | tail: op 8
"""

import math
import sys

sys.path.insert(0, "/opt/trn_rl_repo")


import numpy as np

import concourse.bass as bass
import concourse.mybir as mybir
import concourse.tile as tile
from concourse import bacc
from concourse.bass import ds, ts
from concourse.bass_utils import run_bass_kernel_spmd

F32 = mybir.dt.float32
BF16 = mybir.dt.bfloat16
I16 = mybir.dt.int16
EXP = mybir.ActivationFunctionType.Exp
MULT = mybir.AluOpType.mult
ADD = mybir.AluOpType.add

B, C, L, H = 4, 512, 2048, 8
D = C // H
LQ = L // 2
SCALE = float(D) ** -0.5
NCORES = 8
NL = L // 128  # 16 key chunks
NCH = C // 128  # 4 channel chunks

ESB_BUFS = 38  # j0's 32 exp tiles buffered through phase B + j1 hoist + slack

# Schraudolph exp in bf16 bit patterns (HW rounds to nearest)
SCHRAU_A = float(SCALE * (2.0**7) / math.log(2.0))
SCHRAU_B = float(127.0 * 2.0**7 - 0.0438 * 2.0**7)


def build_nc():
    nc = bacc.Bacc("TRN2", target_bir_lowering=False, debug=False)

    xb_d = nc.dram_tensor("xb", [C, L], BF16, kind="ExternalInput")
    wqT_d = nc.dram_tensor("wqT", [128, NCH, C], BF16, kind="ExternalInput")
    wkT_d = nc.dram_tensor("wkT", [128, NCH, C], BF16, kind="ExternalInput")
    wvT_d = nc.dram_tensor("wvT", [128, NCH, C], BF16, kind="ExternalInput")
    woT_d = nc.dram_tensor("woT", [128, NCH, C], BF16, kind="ExternalInput")
    ident_d = nc.dram_tensor("ident", [128, 128], BF16, kind="ExternalInput")
    y_d = nc.dram_tensor("y", [C, LQ], F32, kind="ExternalOutput")

    with tile.TileContext(nc) as tc:
        with tc.tile_pool(name="persist", bufs=1) as pp:
            kt_t = pp.tile([128, NCH, L], BF16)
            qt_t = pp.tile([128, NCH, LQ], BF16)
            vaug_t = pp.tile([128, NL, H * 65], BF16)
            ot_t = pp.tile([128, NCH, LQ], BF16)
            wo_t = pp.tile([128, NCH, C], BF16)
            id_t = pp.tile([128, 128], BF16)
            nc.sync.dma_start(out=id_t, in_=ident_d.ap())
            # xt persists to the end: its first LQ columns are the bf16
            # residual, injected into the out-proj PSUM via identity matmul
            xt = pp.tile([128, NCH, L], BF16)
            nc.vector.memset(
                vaug_t.rearrange("p lc (h u) -> p lc h u", u=65)[:, :, :, 64], 1.0
            )

            with tc.tile_pool(name="proj_sb", bufs=1) as xp:
                wq_t = xp.tile([128, NCH, C], BF16)
                wk_t = xp.tile([128, NCH, C], BF16)
                wv_t = xp.tile([128, NCH, C], BF16)
                # x is the critical early load: give it two DMA queues; wv
                # rides behind x on those queues (V-proj needs all 4 chunks)
                # while wk/wq go on gpsimd
                for kc in range(NCH):
                    eng = (nc.sync, nc.scalar)[kc % 2]
                    eng.dma_start(
                        out=xt[:, kc, :],
                        in_=xb_d.ap().rearrange("(c p) l -> p c l", p=128)[:, kc, :],
                    )
                    nc.gpsimd.dma_start(out=wk_t[:, kc, :], in_=wkT_d.ap()[:, kc, :])
                for kc in range(NCH):
                    nc.gpsimd.dma_start(out=wq_t[:, kc, :], in_=wqT_d.ap()[:, kc, :])
                for kc in range(NCH):
                    eng = (nc.sync, nc.scalar)[kc % 2]
                    eng.dma_start(out=wv_t[:, kc, :], in_=wvT_d.ap()[:, kc, :])
                with tc.tile_pool(name="sc_ps", bufs=2, space="PSUM") as scps, \
                     tc.tile_pool(name="ex_sb", bufs=ESB_BUFS) as esb, \
                     tc.tile_pool(name="nrm_sb", bufs=2) as nsb:

                    ex_store = {}

                    def emit_scores(j, lc):
                        # sc_a first: its exp runs on the slower engine (DVE),
                        # so when sc_a's bank frees, sc_b's already has — the
                        # scheduler then compiles a/b matmuls adjacent and the
                        # PE overlaps them (disjoint row groups).
                        sc_a = scps.tile([128, LQ], F32, tag="sc")
                        sc_b = scps.tile([128, LQ], F32, tag="sc")
                        for nq in range(LQ // 512):
                            nc.tensor.matmul(
                                sc_a[:, ts(nq, 512)],
                                kt_t[0:64, j, ts(lc, 128)],
                                qt_t[0:64, j, ts(nq, 512)],
                                start=True,
                                stop=True,
                            )
                            nc.tensor.matmul(
                                sc_b[:, ts(nq, 512)],
                                kt_t[64:128, j, ts(lc, 128)],
                                qt_t[64:128, j, ts(nq, 512)],
                                start=True,
                                stop=True,
                            )
                        ex_a = esb.tile([128, LQ], BF16, tag="ex")
                        # two halves: sc_a frees deterministically AFTER sc_b
                        # (2x760 DVE > 1171 ACT) without serializing the next
                        # chunk's scores behind a monolithic 1.5us DVE op
                        for h in range(2):
                            nc.vector.tensor_scalar(
                                ex_a[:, :].bitcast(I16)[:, ts(h, 512)],
                                sc_a[:, ts(h, 512)],
                                SCHRAU_A,
                                SCHRAU_B,
                                MULT,
                                ADD,
                            )
                        ex_b = esb.tile([128, LQ], BF16, tag="ex")
                        nc.scalar.activation(ex_b[:], sc_b[:], EXP, scale=SCALE)
                        ex_store[(j, lc)] = (ex_a, ex_b)

                    # ------------ phase B+C: V-proj || scores/exp j0 ||
                    # K/Q proj mc=1..3 (kq2 groups spread across the lc loop;
                    # K evicts on DVE, Q evicts on scalar to balance queues)
                    vsc = vaug_t.rearrange("p lc (h u) -> p lc h u", u=65)
                    # mc=0's groups lead (j0 scores need them); the rest spread
                    # through the lc loop
                    kq2_groups = [(0, 0, False), (0, 0, True), (0, 1, True),
                                  (0, 1, False), (0, 2, False), (0, 3, False)]
                    kq2_groups += [(mc, grp, False) for mc in (1, 2, 3) for grp in range(4)]
                    kq2_groups += [(mc, grp, True) for mc in (1, 2, 3) for grp in range(2)]
                    with tc.tile_pool(name="kq2_ps", bufs=2, space="PSUM") as kq2, \
                         tc.tile_pool(name="vp_ps", bufs=2, space="PSUM") as vps:

                        def emit_kq2(idx):
                            mc, grp, is_q = kq2_groups[idx]
                            w_t, dst = (wq_t, qt_t) if is_q else (wk_t, kt_t)
                            ps = kq2.tile([128, 512], F32, tag="kq2", name=f"kq2_{idx}")
                            for kc in range(NCH):
                                nc.tensor.matmul(
                                    ps[:, :],
                                    w_t[:, kc, ts(mc, 128)],
                                    xt[:, kc, ts(grp, 512)],
                                    start=(kc == 0),
                                    stop=(kc == NCH - 1),
                                )
                            if is_q:
                                nc.scalar.copy(dst[:, mc, ts(grp, 512)], ps[:, :])
                            else:
                                nc.vector.tensor_copy(dst[:, mc, ts(grp, 512)], ps[:, :])

                        # phase A equivalent: mc=0 groups up front
                        for gidx in range(6):
                            emit_kq2(gidx)
                        gidx = 6
                        for lc in range(NL):
                            ps = vps.tile([128, 512], F32, tag="vp")
                            for kc in range(NCH):
                                nc.tensor.matmul(
                                    ps[:, :],
                                    xt[:, kc, ts(lc, 128)],
                                    wv_t[:, kc, :],
                                    start=(kc == 0),
                                    stop=(kc == NCH - 1),
                                )
                            # bv is all-zero per the problem spec: plain copy
                            nc.scalar.copy(
                                vsc[:, lc, :, 0:64],
                                ps[:, :].rearrange("p (h u) -> p h u", u=64),
                            )
                            emit_scores(0, lc)
                            while gidx < 6 + (lc + 1) * (len(kq2_groups) - 6) // NL:
                                emit_kq2(gidx)
                                gidx += 1
                        while gidx < len(kq2_groups):
                            emit_kq2(gidx)
                            gidx += 1

                    # wo is needed only by the out-proj epilogue; load it
                    # after the hot x/weight DMAs so it doesn't steal HBM
                    # bandwidth from the phase A/B ramp
                    nc.scalar.dma_start(out=wo_t, in_=woT_d.ap())

                    # ------------ phase D: attention ------------
                    with tc.tile_pool(name="ut_ps", bufs=2, space="PSUM") as utps:

                        def emit_evict(j, ut_a, ut_b):
                            # scalar eviction frees the UT PSUM banks ~1us
                            # after the last AV; the rest of the normalize
                            # (recip/broadcast/mul) runs off the critical path
                            uts_pair = []
                            for ut in (ut_a, ut_b):
                                den1 = nsb.tile([1, LQ], F32, tag="d1")
                                nc.scalar.copy(den1[:, :], ut[64:65, :])
                                uts = nsb.tile([64, LQ], F32, tag="uts")
                                nc.scalar.copy(uts[:, :], ut[0:64, :])
                                uts_pair.append((uts, den1))
                            return uts_pair

                        def emit_normalize(j, uts_pair):
                            for hi, (uts, den1) in enumerate(uts_pair):
                                inv1 = nsb.tile([1, LQ], F32, tag="i1")
                                nc.vector.reciprocal_approx_fast(
                                    inv1[:, :], den1[:, :]
                                )
                                invb = nsb.tile([64, LQ], F32, tag="invb")
                                nc.gpsimd.partition_broadcast(invb[:, :], inv1[:, :])
                                nc.vector.tensor_mul(
                                    ot_t[64 * hi : 64 * hi + 64, j, :],
                                    uts[:, :],
                                    invb[:, :],
                                )

                        pending = None  # (j, uts_pair) awaiting normalize
                        for j in range(H // 2):
                            ut_a = utps.tile([128, LQ], F32, tag="ut", name=f"uta{j}")
                            ut_b = utps.tile([128, LQ], F32, tag="ut", name=f"utb{j}")

                            def emit_av(j, pl, ut_a=ut_a, ut_b=ut_b):
                                ex_a, ex_b = ex_store[(j, pl)]
                                for hh, ut, ex in (
                                    (2 * j, ut_a, ex_a),
                                    (2 * j + 1, ut_b, ex_b),
                                ):
                                    va = vaug_t[:, pl, ds(hh * 65, 65)]
                                    for nq in range(LQ // 512):
                                        nc.tensor.matmul(
                                            ut[0:65, ts(nq, 512)],
                                            va,
                                            ex[:, ts(nq, 512)],
                                            start=(pl == 0),
                                            stop=(pl == NL - 1),
                                        )

                            if j == 0:
                                # hoist j1's first scores so their exps land
                                # during the AV burst and j1 starts hot
                                for lc in range(3):
                                    emit_scores(1, lc)
                                # scores/exp j0 ran in phase B; drain backlog
                                for pl in range(NL):
                                    emit_av(0, pl)
                            else:
                                for lc in range(NL + 1):
                                    if lc < NL and not (j == 1 and lc < 3):
                                        emit_scores(j, lc)
                                    if lc == 3 and pending is not None:
                                        # normalize of j-1, emitted after the
                                        # first few scores so the DVE queue
                                        # feeds the tensor engine first
                                        emit_normalize(*pending)
                                        pending = None
                                    if lc > 0:
                                        emit_av(j, lc - 1)
                            pending = (j, emit_evict(j, ut_a, ut_b))
                        emit_normalize(*pending)

            # ---------------- out projection + residual ----------------
            with tc.tile_pool(name="op_ps", bufs=1, space="PSUM") as opps, \
                 tc.tile_pool(name="y_sb", bufs=2) as ysb:
                pss = [
                    opps.tile([128, 2, 512], F32, tag=f"op{mc}", name=f"op{mc}")
                    for mc in range(NCH)
                ]
                # bf16 identity matmuls preload the residual (xt's first LQ
                # cols) into PSUM early; the tail is then just copies + DMA
                for mc in range(NCH):
                    for nq in range(LQ // 512):
                        nc.tensor.matmul(
                            pss[mc][:, nq, :],
                            id_t[:, :],
                            xt[:, mc, ts(nq, 512)],
                            start=True,
                            stop=False,
                            skip_group_check=True,
                        )
                for kc in range(NCH):
                    for mc in range(NCH):
                        for nq in range(LQ // 512):
                            nc.tensor.matmul(
                                pss[mc][:, nq, :],
                                wo_t[:, kc, ts(mc, 128)],
                                ot_t[:, kc, ts(nq, 512)],
                                start=False,
                                stop=(kc == NCH - 1),
                                skip_group_check=True,
                            )
                for mc in range(NCH):
                    y_t = ysb.tile([128, LQ], F32, tag="y")
                    cp = (nc.scalar.copy, nc.vector.tensor_copy)[mc % 2]
                    cp(y_t[:, :], pss[mc].rearrange("p a b -> p (a b)"))
                    eng = (nc.sync, nc.gpsimd, nc.scalar, nc.sync)[mc]
                    eng.dma_start(
                        out=y_d.ap().rearrange("(c p) l -> p c l", p=128)[:, mc, :],
                        in_=y_t,
                    )

    nc.compile()
    return nc


_NC_CACHE = {}


def _get_nc():
    if "nc" not in _NC_CACHE:
        _NC_CACHE["nc"] = build_nc()
    return _NC_CACHE["nc"]


def kernel(x, Wq, bq, Wk, bk, Wv, bv, Wo, bo, _trace=False, _tmpdir=None):
    import ml_dtypes

    npp = ml_dtypes.bfloat16
    x = np.asarray(x, dtype=np.float32)
    assert np.abs(np.asarray(bq)).max() == 0.0
    assert np.abs(np.asarray(bk)).max() == 0.0
    assert np.abs(np.asarray(bv)).max() == 0.0
    assert np.abs(np.asarray(bo)).max() == 0.0
    nc = _get_nc()

    def _tile_w(w):
        wT = np.asarray(w, np.float32).T.reshape(NCH, 128, C).transpose(1, 0, 2)
        return np.ascontiguousarray(wT).astype(npp)

    shared = {
        "wqT": _tile_w(Wq),
        "wkT": _tile_w(Wk),
        "wvT": _tile_w(Wv),
        "woT": _tile_w(Wo),
        "ident": np.eye(128, dtype=np.float32).astype(npp),
    }
    in_maps = []
    for core in range(NCORES):
        b, half = core // 2, core % 2
        xb = x[b]
        # rotate so this core's query half occupies columns 0:LQ; attention
        # is invariant to key order, and all other uses are column-sliced
        xrot = np.ascontiguousarray(
            np.concatenate(
                [
                    xb[:, half * LQ : (half + 1) * LQ],
                    xb[:, (1 - half) * LQ : (2 - half) * LQ],
                ],
                axis=1,
            )
        )
        m = dict(shared)
        m["xb"] = xrot.astype(npp)
        in_maps.append(m)

    res = run_bass_kernel_spmd(
        nc, in_maps, list(range(NCORES)), trace=_trace, tmpdir=_tmpdir
    )

    y = np.empty((B, C, L), np.float32)
    for core in range(NCORES):
        b, half = core // 2, core % 2
        y[b, :, half * LQ : (half + 1) * LQ] = res.results[core]["y"]
    kernel.last_exec_time_ns = res.exec_time_ns if _trace else None
    return y
